# revision 28
# baseline (speedup 1.0000x reference)
"""GAT link-prediction kernel for 8 Trainium2 NeuronCores (Bass/Tile).

v2 design (vs baseline):
- Layer 2 reformulated: W2 projection commutes with the attention-weighted
  scatter-sum (xl2 = h1 @ W2 is linear), so the per-edge gather pulls 256B
  rows [h1(32)|a_src2~(8)|pad] instead of 768B projected rows; W2 applied
  per dst block after softmax normalization via two K=128 matmuls on the
  transposed accumulator.
- Layer-2 table is built as a per-core 1.6MB shard and replicated by one
  AllGather (replaces the 620us redundant build2 phase).
- Layer 1 gathers 512B xl-only rows; per-edge a_src1 recomputed on DVE
  (mult + reduce against att_src1).
- Per-layer edge bucketing: L1 needs no src-half split (vocab idx < 5120);
  L2 buckets split by SOURCE SLOT RANGE (A=slots 0-24, B=25-48) so the
  A-half table AllGather overlaps layer-1 blocks 25-48; only the B-half
  AG (~50us) is exposed, and L2's A-bucket gathers start before AG-B ends.
- Dst blocks rebalanced across cores (greedy by edge count) to cut padding.
- z packed to 32 f32/row; decode gathers use 256B elements with a parity
  byte-offset trick (even rows from base+0, odd from base+128B).
"""

import numpy as np
import ml_dtypes

import concourse.bass as bass
import concourse.bacc as bacc
import concourse.mybir as mybir
import concourse.tile as tile
from concourse.bass_utils import run_bass_kernel_spmd
from concourse.masks import make_identity

P = 128
NC = 8
N = 50000
V = 5000
VPAD = 5120
EL = 200000
D = 128
HID = 32
OUT = 32
H = 8
NEG = 0.2
NSLOT = 49
NBLK = NC * NSLOT          # 392
RPC = NSLOT * P            # 6272 table rows per core
NROWS = NC * RPC           # 50176
HALFROW = NROWS // 2       # 25088
ELC = EL // NC
F32 = mybir.dt.float32
BF16 = mybir.dt.bfloat16
FP8 = mybir.dt.float8e4
I16 = mybir.dt.int16
EXP = mybir.ActivationFunctionType.Exp
RELU = mybir.ActivationFunctionType.Relu
MULT = mybir.AluOpType.mult
ADD = mybir.AluOpType.add
MAXOP = mybir.AluOpType.max
AXX = mybir.AxisListType.X


def _wrap16(idx_flat):
    n = len(idx_flat)
    assert n % 16 == 0
    w = np.zeros((16, n // 16), np.int16)
    w[np.arange(n) % 16, np.arange(n) // 16] = idx_flat
    return np.tile(w, (8, 1))


def _plan(x, edge_index, eli):
    xs = x.astype(np.int64)
    src = np.concatenate([edge_index[0], np.arange(N)]).astype(np.int64)
    dst = np.concatenate([edge_index[1], np.arange(N)]).astype(np.int64)
    eblk = dst // P
    cntb = np.bincount(eblk, minlength=NBLK)

    # greedy block -> (core, slot) assignment balancing edge counts
    order = np.argsort(-cntb, kind="stable")
    core_of = np.zeros(NBLK, np.int64)
    slot_of = np.zeros(NBLK, np.int64)
    load = np.zeros(NC, np.int64)
    for s in range(NSLOT):
        grp = order[s * 8:(s + 1) * 8]
        gg = grp[np.argsort(-cntb[grp], kind="stable")]
        cores = np.argsort(load, kind="stable")
        for k, b in enumerate(gg):
            core_of[b] = cores[k]
            slot_of[b] = s
            load[cores[k]] += cntb[b]

    # pos-major row order: matches both the [p, (slot e)] shard write and the
    # z AllGather layout (core, pos, slot)
    nodes = np.arange(N)
    tabrow = (core_of[nodes // P] * RPC + (nodes % P) * NSLOT
              + slot_of[nodes // P])

    ecore = core_of[eblk]
    eslot = slot_of[eblk]
    dpos = dst % P

    # ---- layer 1 buckets: (core, slot), sorted by vocab id ----
    o1 = np.lexsort((xs[src], eslot, ecore))
    cnt1 = np.zeros((NC, NSLOT), np.int64)
    np.add.at(cnt1, (ecore, eslot), 1)
    CH1 = np.maximum(1, -(-cnt1.max(axis=0) // P))
    ch1_off = np.concatenate([[0], np.cumsum(CH1)])[:-1]
    TOTCH1 = int(CH1.sum())
    TOTE1 = TOTCH1 * P

    # ---- layer 2 buckets: (core, slot, A/B) split by SOURCE slot range ----
    # A = src slots 0-30 (table ready after L1 block 30), B = slots 31-48;
    # A capped at 31 slots so 8*31*128 = 31744 stays within int16
    c_src = slot_of[src // P]
    p_src = src % P
    r_src = core_of[src // P]
    half = (c_src >= 31).astype(np.int64)
    trow2 = np.where(half == 0,
                     r_src * 3968 + p_src * 31 + c_src,
                     r_src * 2304 + p_src * 18 + (c_src - 31))
    o2 = np.lexsort((trow2, half, eslot, ecore))
    cnt2 = np.zeros((NC, NSLOT, 2), np.int64)
    np.add.at(cnt2, (ecore, eslot, half), 1)
    CH2 = -(-cnt2.max(axis=0) // P)
    ch2_off = np.zeros((NSLOT, 2), np.int64)
    run = 0
    for s in range(NSLOT):
        ch2_off[s, 0] = run
        run += CH2[s, 0]
        ch2_off[s, 1] = run
        run += CH2[s, 1]
    TOTCH2 = int(run)
    TOTE2 = TOTCH2 * P

    # flat start offsets of each core's buckets in the sorted order
    per_core = []
    start1 = np.zeros((NC, NSLOT), np.int64)
    pos = 0
    for c in range(NC):
        for s in range(NSLOT):
            start1[c, s] = pos
            pos += cnt1[c, s]
    start2 = np.zeros((NC, NSLOT, 2), np.int64)
    pos = 0
    for c in range(NC):
        for s in range(NSLOT):
            for h in range(2):
                start2[c, s, h] = pos
                pos += cnt2[c, s, h]

    src1 = src[o1]
    dst1p = dpos[o1]
    trow2s = trow2[o2]
    dst2p = dpos[o2]
    for c in range(NC):
        idx1 = np.zeros(TOTE1, np.int64)
        dl1 = np.full(TOTE1, -1, np.int64)
        for s in range(NSLOT):
            nr = int(cnt1[c, s])
            s0 = int(start1[c, s])
            o0 = int(ch1_off[s]) * P
            idx1[o0:o0 + nr] = xs[src1[s0:s0 + nr]]
            dl1[o0:o0 + nr] = dst1p[s0:s0 + nr]
        idx2 = np.zeros(TOTE2, np.int64)
        dl2 = np.full(TOTE2, -1, np.int64)
        for s in range(NSLOT):
            for h in range(2):
                nr = int(cnt2[c, s, h])
                s0 = int(start2[c, s, h])
                o0 = int(ch2_off[s, h]) * P
                idx2[o0:o0 + nr] = trow2s[s0:s0 + nr]
                dl2[o0:o0 + nr] = dst2p[s0:s0 + nr]
        per_core.append((idx1, dl1, idx2, dl2))

    # ---- decode plan: 4 parity groups ----
    z0 = tabrow[eli[0]]
    z1 = tabrow[eli[1]]
    dec_grp_chunks = np.zeros(4, np.int64)
    dec_core = []
    for c in range(NC):
        a = z0[c * ELC:(c + 1) * ELC]
        b = z1[c * ELC:(c + 1) * ELC]
        grp = (a & 1) * 2 + (b & 1)
        order_d = np.argsort(grp, kind="stable")
        gi0, gi1, gch, gsz, perm = [], [], [], [], []
        for g in range(4):
            m = grp[order_d] == g
            ids0 = (a[order_d][m]) >> 1
            ids1 = (b[order_d][m]) >> 1
            pidx = order_d[m]
            gsz.append(len(ids0))
            npad = (-len(ids0)) % P
            ids0 = np.concatenate([ids0, np.zeros(npad, np.int64)])
            ids1 = np.concatenate([ids1, np.zeros(npad, np.int64)])
            pidx = np.concatenate([pidx, np.full(npad, -1)])
            gch.append(len(ids0) // P)
            gi0.append(ids0)
            gi1.append(ids1)
            perm.append(pidx)
        dec_grp_chunks = np.maximum(dec_grp_chunks, gch)
        dec_core.append((gi0, gi1, gch, gsz, perm))

    meta = dict(CH1=CH1, ch1_off=ch1_off, TOTCH1=TOTCH1, TOTE1=TOTE1,
                CH2=CH2, ch2_off=ch2_off, TOTCH2=TOTCH2, TOTE2=TOTE2,
                dec_grp_chunks=[int(v) for v in dec_grp_chunks],
                core_of=core_of, slot_of=slot_of, tabrow=tabrow,
                cnt1=cnt1, cnt2=cnt2)
    return per_core, dec_core, meta


def _call_plan(meta):
    """Gather-call order shared by device program and host count tables.
    Entries: (kind, slot, half, chunk_start, n_chunks)."""
    CH1 = meta["CH1"]
    CH2 = meta["CH2"]
    dgc = meta["dec_grp_chunks"]
    calls = []
    for b in range(NSLOT):
        C = int(CH1[b])
        for s in range(0, C, 8):
            calls.append(("l1", b, 0, s, min(8, C - s)))
    for b in range(NSLOT):
        for hh in (0, 1):
            for s in range(0, int(CH2[b, hh]), 8):
                calls.append(("l2", b, hh, s, min(8, int(CH2[b, hh]) - s)))
    for gi in range(4):
        for s in range(0, dgc[gi], 8):
            calls.append(("d0", gi, 0, s, min(8, dgc[gi] - s)))
            calls.append(("d1", gi, 1, s, min(8, dgc[gi] - s)))
    return calls


def _build_nc(meta):
    CH1 = meta["CH1"]
    ch1_off = meta["ch1_off"]
    TOTE1 = meta["TOTE1"]
    CH2 = meta["CH2"]
    ch2_off = meta["ch2_off"]
    TOTE2 = meta["TOTE2"]
    dgc = meta["dec_grp_chunks"]
    DGC = sum(dgc)
    CM1 = int(CH1.max())
    CM2 = int(CH2.sum(axis=1).max())
    CMX = max(CM1, CM2)

    nc = bacc.Bacc("TRN2", target_bir_lowering=False, debug=False,
                   num_devices=NC, num_swdge_queues=4)

    t_embT = nc.dram_tensor("embT", [D, VPAD], BF16, kind="ExternalInput")
    t_w1x = nc.dram_tensor("w1x", [D, 272], BF16, kind="ExternalInput")
    t_b1 = nc.dram_tensor("b1", [P, HID], F32, kind="ExternalInput")
    t_b2 = nc.dram_tensor("b2", [P, OUT], F32, kind="ExternalInput")
    t_w2s = nc.dram_tensor("w2s", [HID, 8], BF16, kind="ExternalInput")
    t_w2d = nc.dram_tensor("w2d", [HID, 8], BF16, kind="ExternalInput")
    t_w2stk = nc.dram_tensor("w2stk", [P, 64], F32, kind="ExternalInput")
    t_idx1 = nc.dram_tensor("idx1", [P, TOTE1 // 16], I16, kind="ExternalInput")
    t_idx2 = nc.dram_tensor("idx2", [P, TOTE2 // 16], I16, kind="ExternalInput")
    t_xn = nc.dram_tensor("xn", [P, RPC // 16], I16, kind="ExternalInput")
    t_oh1 = nc.dram_tensor("oh1", [P, TOTE1], FP8, kind="ExternalInput")
    t_oht1 = nc.dram_tensor("oht1", [P, TOTE1], FP8, kind="ExternalInput")
    t_oh2 = nc.dram_tensor("oh2", [P, TOTE2], FP8, kind="ExternalInput")
    t_oht2 = nc.dram_tensor("oht2", [P, TOTE2], FP8, kind="ExternalInput")
    calls = _call_plan(meta)
    NCALL = len(calls)
    t_gc = nc.dram_tensor("gc", [P, NCALL], mybir.dt.int32, kind="ExternalInput")
    t_di0 = nc.dram_tensor("di0", [P, DGC * 8], I16, kind="ExternalInput")
    t_di1 = nc.dram_tensor("di1", [P, DGC * 8], I16, kind="ExternalInput")
    t_out = nc.dram_tensor("out", [P, DGC], F32, kind="ExternalOutput")

    rr = [0]

    def nextq():
        rr[0] = (rr[0] + 1) % 4
        return rr[0]

    with tile.TileContext(nc) as tc:
        with (
            tc.tile_pool(name="dram", bufs=1, space="DRAM") as dp,
            tc.tile_pool(name="const", bufs=1) as cp,
            tc.tile_pool(name="sm", bufs=6) as sm,
            tc.tile_pool(name="psA", bufs=3, space="PSUM") as psA,
            tc.tile_pool(name="psB", bufs=2, space="PSUM") as psB,
            tc.tile_pool(name="psC", bufs=1, space="PSUM") as psC,
            tc.tile_pool(name="psD", bufs=2, space="PSUM") as psD,
        ):
            tab1 = dp.tile([VPAD, 384], BF16)
            tab2shA = dp.tile([3968, P], BF16)
            tab2shB = dp.tile([2304, P], BF16)
            tabAfull = dp.tile([NC * 3968, P], BF16, addr_space="Shared")
            tabBfull = dp.tile([NC * 2304, P], BF16, addr_space="Shared")
            zloc = dp.tile([P, NSLOT * 32], F32)
            zall = dp.tile([1026, NSLOT * 32], F32, addr_space="Shared")

            # ---------- constants ----------
            ident = cp.tile([P, P], F32)
            make_identity(nc, ident[:])
            identb = cp.tile([P, P], BF16)
            nc.vector.tensor_copy(out=identb[:], in_=ident[:])
            embT_sb = cp.tile([D, VPAD], BF16)
            nc.sync.dma_start(out=embT_sb[:], in_=t_embT[:, :])
            w1x_sb = cp.tile([D, 272], BF16)
            nc.sync.dma_start(out=w1x_sb[:], in_=t_w1x[:, :])
            b1_sb = cp.tile([P, HID], F32)
            nc.sync.dma_start(out=b1_sb[:], in_=t_b1[:, :])
            b2_sb = cp.tile([P, OUT], F32)
            nc.sync.dma_start(out=b2_sb[:], in_=t_b2[:, :])
            w2s_sb = cp.tile([HID, 8], BF16)
            nc.sync.dma_start(out=w2s_sb[:], in_=t_w2s[:, :])
            w2d_sb = cp.tile([HID, 8], BF16)
            nc.sync.dma_start(out=w2d_sb[:], in_=t_w2d[:, :])
            w2stk_sb = cp.tile([P, 64], F32)
            nc.sync.dma_start(out=w2stk_sb[:], in_=t_w2stk[:, :])
            xn_sb = cp.tile([P, RPC // 16], I16)
            nc.sync.dma_start(out=xn_sb[:], in_=t_xn[:, :])
            an1_sb = cp.tile([P, NSLOT, 8], BF16)
            an2_sb = cp.tile([P, NSLOT, 8], BF16)
            h1t_sb = cp.tile([HID, RPC], BF16)
            t2row_sb = cp.tile([P, NSLOT, P], BF16)
            z_sb = cp.tile([P, NSLOT, 32], F32)
            gc_sb = cp.tile([P, NCALL], mybir.dt.int32)
            nc.sync.dma_start(out=gc_sb[:], in_=t_gc[:, :])
            greg = nc.gpsimd.alloc_register("gcnt")
            kk = [0]

            def gather_exact(out_ap, in_ap, idxs_ap, n_static, elem):
                kk[0] += 1
                nc.gpsimd.dma_gather(out_ap, in_ap, idxs_ap, n_static,
                                     n_static, elem, queue_num=nextq())

            di0_sb = cp.tile([P, DGC * 8], I16)
            nc.sync.dma_start(out=di0_sb[:], in_=t_di0[:, :])
            di1_sb = cp.tile([P, DGC * 8], I16)
            nc.sync.dma_start(out=di1_sb[:], in_=t_di1[:, :])
            res = cp.tile([P, DGC], F32)

            nc.vector.memset(t2row_sb[:, :, 40:P], 0)

            # ---------- build tab1: xl1 = emb @ W1, 512B rows ----------
            with nc.named_scope("build1"):
                with (
                    tc.tile_pool(name="b1p", bufs=3) as bp,
                ):
                    for tv in range(VPAD // P):
                        acc = psB.tile([P, 288], F32, space="PSUM", tag="big")
                        nc.tensor.matmul(out=acc[:, 0:272],
                                         lhsT=embT_sb[:, tv * P:(tv + 1) * P],
                                         rhs=w1x_sb[:], start=True, stop=True)
                        ob = bp.tile([P, 272], BF16, tag="obf")
                        nc.scalar.copy(out=ob[:], in_=acc[:, 0:272])
                        nc.sync.dma_start(out=tab1[tv * P:(tv + 1) * P, 0:272],
                                          in_=ob[:])

                    # an1: per-node a_dst1 from the table's att_dst column
                    for c0 in range(0, NSLOT, 8):
                        cw = min(8, NSLOT - c0)
                        gt = bp.tile([P, 8, 384], BF16, tag="ang")
                        nc.gpsimd.dma_gather(gt[:, 0:cw, :], tab1[:, :],
                                             xn_sb[:, c0 * 8:(c0 + cw) * 8],
                                             cw * P, cw * P, 384,
                                             queue_num=nextq())
                        nc.vector.tensor_copy(out=an1_sb[:, c0:c0 + cw, :],
                                              in_=gt[:, 0:cw, 264:272])

            # ---------- layer 1 ----------
            with nc.named_scope("layer1"):
                with (
                    tc.tile_pool(name="l1g", bufs=3) as gp,
                ):
                    for _ in range(4):
                        gz = gp.tile([P, CM1, 384], BF16, tag="g1", bufs=4)
                        nc.vector.memset(gz[:], 0)
                    for b in range(NSLOT):
                        C = int(CH1[b])
                        base = int(ch1_off[b])
                        idxb = sm.tile([P, CM1 * 8], I16, tag="idxb")
                        nc.sync.dma_start(out=idxb[:, 0:C * 8],
                                          in_=t_idx1[:, base * 8:(base + C) * 8])
                        g1 = gp.tile([P, CM1, 384], BF16, tag="g1", bufs=4)
                        for s in range(0, C, 8):
                            cw = min(8, C - s)
                            gather_exact(g1[:, s:s + cw, :], tab1[:, :],
                                         idxb[:, s * 8:(s + cw) * 8],
                                         cw * P, 384)
                        oh = gp.tile([P, CM1 * P], FP8, tag="oh", bufs=4)
                        nc.sync.dma_start(out=oh[:, 0:C * P],
                                          in_=t_oh1[:, base * P:(base + C) * P])
                        oht = gp.tile([P, CM1 * P], FP8, tag="oht", bufs=4)
                        nc.sync.dma_start(out=oht[:, 0:C * P],
                                          in_=t_oht1[:, base * P:(base + C) * P])
                        # a_dst per edge via transposed one-hot
                        adp = psC.tile([P, CMX * 8], F32, space="PSUM", tag="adp")
                        for j in range(C):
                            nc.tensor.matmul(out=adp[:, j * 8:(j + 1) * 8],
                                             lhsT=oht[:, j * P:(j + 1) * P],
                                             rhs=an1_sb[:, b, :],
                                             start=True, stop=True)
                        te = sm.tile([P, CM1, 8], F32, tag="te")
                        nc.vector.tensor_tensor(
                            out=te[:, 0:C, :], in0=g1[:, 0:C, 256:264],
                            in1=adp[:, 0:C * 8].rearrange("p (c a) -> p c a", a=8),
                            op=ADD)
                        e1 = sm.tile([P, CM1, 8], F32, tag="e1")
                        nc.scalar.activation(out=e1[:, 0:C, :], in_=te[:, 0:C, :],
                                             func=EXP)
                        e2 = sm.tile([P, CM1, 8], F32, tag="e2")
                        nc.scalar.activation(out=e2[:, 0:C, :], in_=te[:, 0:C, :],
                                             func=EXP, scale=NEG)
                        ee = sm.tile([P, CM1, 8], BF16, tag="ee")
                        nc.vector.tensor_tensor(out=ee[:, 0:C, :],
                                                in0=e1[:, 0:C, :],
                                                in1=e2[:, 0:C, :], op=MAXOP)
                        # weighted messages in place; ee into cols 256:264
                        eex = gp.tile([P, CM1, 256], BF16, tag="eex", bufs=2)
                        nc.scalar.copy(
                            out=eex[:, 0:C, :].rearrange(
                                "p c (h o) -> p c h o", h=H),
                            in_=ee[:, 0:C, :].to_broadcast([P, C, 8, HID]))
                        nc.vector.tensor_tensor(
                            out=g1[:, 0:C, 0:256], in0=g1[:, 0:C, 0:256],
                            in1=eex[:, 0:C, :], op=MULT)
                        nc.vector.tensor_copy(out=g1[:, 0:C, 256:264],
                                              in_=ee[:, 0:C, :])
                        acc = psA.tile([P, 264], F32, space="PSUM", tag="acc")
                        for j in range(C):
                            nc.tensor.matmul(out=acc[:],
                                             lhsT=oh[:, j * P:(j + 1) * P],
                                             rhs=g1[:, j, 0:264],
                                             start=(j == 0), stop=(j == C - 1))
                        rec = sm.tile([P, 8], F32, tag="rec")
                        nc.vector.tensor_scalar(out=rec[:], in0=acc[:, 256:264],
                                                scalar1=8.0, scalar2=1e-30,
                                                op0=MULT, op1=ADD)
                        nc.vector.reciprocal(out=rec[:], in_=rec[:])
                        hs = sm.tile([P, 256], F32, tag="hs")
                        nc.vector.tensor_tensor(
                            out=hs[:].rearrange("p (h o) -> p h o", h=H),
                            in0=acc[:, 0:256].rearrange("p (h o) -> p h o", h=H),
                            in1=rec[:].to_broadcast([P, H, HID]), op=MULT)
                        hsum = sm.tile([P, 32], F32, tag="hsum")
                        nc.vector.tensor_reduce(
                            out=hsum[:],
                            in_=hs[:].rearrange("p (h o) -> p o h", h=H),
                            axis=AXX, op=ADD)
                        nc.vector.tensor_add(out=hsum[:], in0=hsum[:],
                                             in1=b1_sb[:])
                        nc.scalar.activation(out=hsum[:], in_=hsum[:],
                                             func=RELU)
                        # h1 row (bf16) into the tab2 shard staging tile
                        nc.scalar.copy(out=t2row_sb[:, b, 0:32],
                                       in_=hsum[:])
                        # transposed h1 for the per-node attention projections
                        tp = psB.tile([P, 288], F32, space="PSUM", tag="big")
                        nc.tensor.transpose(out=tp[0:HID, 0:P],
                                            in_=hsum[:],
                                            identity=ident[:])
                        nc.scalar.copy(out=h1t_sb[:, b * P:(b + 1) * P],
                                       in_=tp[0:HID, 0:P])
                        a2p = psD.tile([P, 32], F32, space="PSUM", tag="small")
                        nc.tensor.matmul(out=a2p[:, 0:8],
                                         lhsT=h1t_sb[:, b * P:(b + 1) * P],
                                         rhs=w2s_sb[:], start=True, stop=True)
                        nc.tensor.matmul(out=a2p[:, 8:16],
                                         lhsT=h1t_sb[:, b * P:(b + 1) * P],
                                         rhs=w2d_sb[:], start=True, stop=True)
                        nc.scalar.copy(out=t2row_sb[:, b, 32:40],
                                        in_=a2p[:, 0:8])
                        nc.scalar.copy(out=an2_sb[:, b, :],
                                        in_=a2p[:, 8:16])
                        if b == 30:
                            # A-half table ready: ship + allgather it while
                            # layer 1 keeps running on slots 25-48
                            nc.sync.dma_start(
                                out=tab2shA[:, :].rearrange(
                                    "(p c) e -> p (c e)", p=P),
                                in_=t2row_sb[:, 0:31, :].rearrange(
                                    "p c e -> p (c e)"))
                            nc.gpsimd.collective_compute(
                                "AllGather", mybir.AluOpType.bypass,
                                replica_groups=[list(range(NC))],
                                ins=[tab2shA[:, :].opt()],
                                outs=[tabAfull[:, :].opt()])
                    nc.sync.dma_start(
                        out=tab2shB[:, :].rearrange("(p c) e -> p (c e)", p=P),
                        in_=t2row_sb[:, 31:NSLOT, :].rearrange("p c e -> p (c e)"))

            # ---------- allgather B-half ----------
            nc.gpsimd.collective_compute(
                "AllGather", mybir.AluOpType.bypass,
                replica_groups=[list(range(NC))],
                ins=[tab2shB[:, :].opt()], outs=[tabBfull[:, :].opt()])

            # ---------- layer 2 ----------
            with nc.named_scope("layer2"):
                with (
                    tc.tile_pool(name="l2g", bufs=4) as gp2,
                    tc.tile_pool(name="l2r", bufs=3) as rp2,
                ):
                    for _ in range(4):
                        gz2 = gp2.tile([P, CM2, P], BF16, tag="g2")
                        nc.vector.memset(gz2[:], 0)
                    tabs = (tabAfull[:, :], tabBfull[:, :])
                    for b in range(NSLOT):
                        c0h = [int(ch2_off[b, 0]), int(ch2_off[b, 1])]
                        cws = [int(CH2[b, 0]), int(CH2[b, 1])]
                        C = cws[0] + cws[1]
                        base = c0h[0]
                        idxb = sm.tile([P, CM2 * 8], I16, tag="idxb2")
                        nc.sync.dma_start(out=idxb[:, 0:C * 8],
                                          in_=t_idx2[:, base * 8:(base + C) * 8])
                        g2 = gp2.tile([P, CM2, P], BF16, tag="g2")
                        for hh in (0, 1):
                            off = c0h[hh] - base
                            for s in range(0, cws[hh], 8):
                                cw = min(8, cws[hh] - s)
                                gather_exact(
                                    g2[:, off + s:off + s + cw, :], tabs[hh],
                                    idxb[:, (off + s) * 8:(off + s + cw) * 8],
                                    cw * P, P)
                        oh = gp2.tile([P, CM2 * P], FP8, tag="oh2")
                        nc.sync.dma_start(out=oh[:, 0:C * P],
                                          in_=t_oh2[:, base * P:(base + C) * P])
                        oht = gp2.tile([P, CM2 * P], FP8, tag="oht2")
                        nc.sync.dma_start(out=oht[:, 0:C * P],
                                          in_=t_oht2[:, base * P:(base + C) * P])
                        adp = psC.tile([P, CMX * 8], F32, space="PSUM", tag="adp")
                        for j in range(C):
                            nc.tensor.matmul(out=adp[:, j * 8:(j + 1) * 8],
                                             lhsT=oht[:, j * P:(j + 1) * P],
                                             rhs=an2_sb[:, b, :],
                                             start=True, stop=True)
                        te = sm.tile([P, CM2, 8], F32, tag="te2")
                        nc.vector.tensor_tensor(
                            out=te[:, 0:C, :], in0=g2[:, 0:C, 32:40],
                            in1=adp[:, 0:C * 8].rearrange("p (c a) -> p c a", a=8),
                            op=ADD)
                        e1 = sm.tile([P, CM2, 8], F32, tag="e12")
                        nc.scalar.activation(out=e1[:, 0:C, :], in_=te[:, 0:C, :],
                                             func=EXP)
                        e2 = sm.tile([P, CM2, 8], F32, tag="e22")
                        nc.scalar.activation(out=e2[:, 0:C, :], in_=te[:, 0:C, :],
                                             func=EXP, scale=NEG)
                        ee = sm.tile([P, CM2, 8], BF16, tag="ee2")
                        nc.vector.tensor_tensor(out=ee[:, 0:C, :],
                                                in0=e1[:, 0:C, :],
                                                in1=e2[:, 0:C, :], op=MAXOP)
                        # rhs: outer product ee[h] x h1[c] per edge
                        r2 = rp2.tile([P, CM2, 264], BF16, tag="r2")
                        eex2 = rp2.tile([P, CM2, 256], BF16, tag="eex2")
                        nc.scalar.copy(
                            out=eex2[:, 0:C, :].rearrange(
                                "p c (h o) -> p c h o", h=H),
                            in_=ee[:, 0:C, :].to_broadcast([P, C, 8, HID]))
                        nc.vector.tensor_tensor(
                            out=r2[:, 0:C, 0:256].rearrange(
                                "p c (h o) -> p c h o", h=H),
                            in0=eex2[:, 0:C, :].rearrange(
                                "p c (h o) -> p c h o", h=H),
                            in1=g2[:, 0:C, None, 0:32].broadcast_to(
                                [P, C, H, HID]),
                            op=MULT)
                        nc.vector.tensor_copy(out=r2[:, 0:C, 256:264],
                                              in_=ee[:, 0:C, :])
                        acc = psA.tile([P, 264], F32, space="PSUM", tag="acc")
                        for j in range(C):
                            nc.tensor.matmul(out=acc[:],
                                             lhsT=oh[:, j * P:(j + 1) * P],
                                             rhs=r2[:, j, :],
                                             start=(j == 0), stop=(j == C - 1))
                        rec = sm.tile([P, 8], F32, tag="rec2")
                        nc.vector.tensor_scalar(out=rec[:], in0=acc[:, 256:264],
                                                scalar1=1.0, scalar2=1e-30,
                                                op0=MULT, op1=ADD)
                        nc.vector.reciprocal(out=rec[:], in_=rec[:])
                        hsn = sm.tile([P, 256], F32, tag="hsn")
                        nc.vector.tensor_tensor(
                            out=hsn[:].rearrange("p (h o) -> p h o", h=H),
                            in0=acc[:, 0:256].rearrange("p (h o) -> p h o", h=H),
                            in1=rec[:].to_broadcast([P, H, HID]), op=MULT)
                        # transpose both halves, project with W2 (1/8 folded)
                        s2t = sm.tile([P, 256], F32, tag="s2t")
                        for k in range(2):
                            tp2 = psB.tile([P, 288], F32, space="PSUM", tag="big")
                            nc.tensor.transpose(out=tp2[:, 0:P],
                                                in_=hsn[:, k * P:(k + 1) * P],
                                                identity=ident[:])
                            nc.scalar.copy(out=s2t[:, k * P:(k + 1) * P],
                                           in_=tp2[:, 0:P])
                        zmm = psD.tile([P, 32], F32, space="PSUM", tag="small")
                        nc.tensor.matmul(out=zmm[:], lhsT=s2t[:, 0:P],
                                         rhs=w2stk_sb[:, 0:32],
                                         start=True, stop=False)
                        nc.tensor.matmul(out=zmm[:], lhsT=s2t[:, P:256],
                                         rhs=w2stk_sb[:, 32:64],
                                         start=False, stop=True)
                        nc.vector.tensor_add(out=z_sb[:, b, :], in0=zmm[:],
                                             in1=b2_sb[:])
                    nc.sync.dma_start(out=zloc[:, :],
                                      in_=z_sb[:].rearrange("p c a -> p (c a)"))

            # ---------- allgather z ----------
            nc.gpsimd.collective_compute(
                "AllGather", mybir.AluOpType.bypass,
                replica_groups=[list(range(NC))],
                ins=[zloc[:, :].opt()], outs=[zall[0:NC * P, :].opt()])

            # ---------- decode ----------
            with nc.named_scope("decode"):
                with tc.tile_pool(name="dg", bufs=4) as dgp:
                    zflat = zall[:, :].rearrange("p f -> (p f)")
                    NPAIR = (1026 * NSLOT * 32) // 64
                    zeven = zflat.rearrange("(r e) -> r e", e=64)
                    zodd = zflat[32:32 + (NPAIR - 1) * 64].rearrange(
                        "(r e) -> r e", e=64)
                    goff = 0
                    for gi in range(4):
                        gch = dgc[gi]
                        v0 = zeven if (gi >> 1) == 0 else zodd
                        v1 = zeven if (gi & 1) == 0 else zodd
                        for s in range(0, gch, 8):
                            cw = min(8, gch - s)
                            z0t = dgp.tile([P, 8, 64], F32, tag="z0")
                            gather_exact(
                                z0t[:, 0:cw, :], v0,
                                di0_sb[:, (goff + s) * 8:(goff + s + cw) * 8],
                                cw * P, 64)
                            z1t = dgp.tile([P, 8, 64], F32, tag="z1")
                            gather_exact(
                                z1t[:, 0:cw, :], v1,
                                di1_sb[:, (goff + s) * 8:(goff + s + cw) * 8],
                                cw * P, 64)
                            nc.vector.tensor_tensor(out=z0t[:, 0:cw, 0:32],
                                                    in0=z0t[:, 0:cw, 0:32],
                                                    in1=z1t[:, 0:cw, 0:32],
                                                    op=MULT)
                            nc.vector.tensor_reduce(
                                out=res[:, goff + s:goff + s + cw],
                                in_=z0t[:, 0:cw, 0:32], axis=AXX, op=ADD)
                        goff += gch
                    nc.sync.dma_start(out=t_out[:, :], in_=res[:])

    nc.compile()
    return nc


def _make_inputs(inputs):
    x = np.asarray(inputs["x"]).astype(np.int64)
    edge_index = np.asarray(inputs["edge_index"]).astype(np.int64)
    eli = np.asarray(inputs["edge_label_index"]).astype(np.int64)
    emb = np.asarray(inputs["emb"]).astype(np.float32)
    W1 = np.asarray(inputs["W1"]).astype(np.float32)
    W2 = np.asarray(inputs["W2"]).astype(np.float32)
    a1s = np.asarray(inputs["att_src1"]).astype(np.float32)
    a1d = np.asarray(inputs["att_dst1"]).astype(np.float32)
    a2s = np.asarray(inputs["att_src2"]).astype(np.float32)
    a2d = np.asarray(inputs["att_dst2"]).astype(np.float32)
    b1 = np.asarray(inputs["b1"]).astype(np.float32).reshape(-1)
    b2 = np.asarray(inputs["b2"]).astype(np.float32).reshape(-1)

    per_core, dec_core, meta = _plan(x, edge_index, eli)
    TOTCH1 = meta["TOTCH1"]
    TOTE1 = meta["TOTE1"]
    TOTCH2 = meta["TOTCH2"]
    TOTE2 = meta["TOTE2"]
    dgc = meta["dec_grp_chunks"]
    DGC = sum(dgc)
    core_of = meta["core_of"]
    slot_of = meta["slot_of"]

    bf = ml_dtypes.bfloat16
    emb_pad = np.zeros((VPAD, D), np.float32)
    emb_pad[:V] = emb
    # W~2s[d,h] = sum_c W2[d, (h,c)] * att_src2[h,c]; same for dst
    W2r = W2.reshape(HID, H, OUT)
    w2s_t = np.einsum("dhc,hc->dh", W2r, a2s)
    w2d_t = np.einsum("dhc,hc->dh", W2r, a2d)
    # W2stack[(h,d), o] with 1/8 head-mean folded in; halves side by side
    w2stk = (W2r.transpose(1, 0, 2).reshape(H * HID, OUT) / 8.0)
    w2stk2 = np.concatenate([w2stk[0:128], w2stk[128:256]], axis=1)  # [128, 64]

    W1r = W1.reshape(D, H, HID)
    w1s_t = np.einsum("dhc,hc->dh", W1r, a1s.reshape(H, HID))
    w1d_t = np.einsum("dhc,hc->dh", W1r, a1d.reshape(H, HID))
    w1x = np.concatenate([W1, w1s_t, w1d_t], axis=1)  # [D, 272]
    common = {
        "embT": emb_pad.T.astype(bf).copy(),
        "w1x": w1x.astype(bf),
        "b1": np.tile(b1, (P, 1)),
        "b2": np.tile(b2, (P, 1)),
        "w2s": w2s_t.astype(bf),
        "w2d": w2d_t.astype(bf),
        "w2stk": w2stk2.astype(np.float32),
    }

    # per-node vocab ids by (slot, pos)
    xn_all = np.zeros((NC, RPC), np.int64)
    nodes = np.arange(N)
    xn_all[core_of[nodes // P], slot_of[nodes // P] * P + nodes % P] = x[nodes]

    in_maps = []
    out_perms = []
    e1i = np.arange(TOTE1)
    e2i = np.arange(TOTE2)
    for c in range(NC):
        idx1, dl1, idx2, dl2 = per_core[c]
        oh1 = np.zeros((P, TOTCH1, P), np.uint8)
        r1 = dl1 >= 0
        oh1[e1i[r1] % P, e1i[r1] // P, dl1[r1]] = 0x38
        oht1 = np.ascontiguousarray(oh1.transpose(2, 1, 0))
        oh2 = np.zeros((P, TOTCH2, P), np.uint8)
        r2 = dl2 >= 0
        oh2[e2i[r2] % P, e2i[r2] // P, dl2[r2]] = 0x38
        oht2 = np.ascontiguousarray(oh2.transpose(2, 1, 0))

        gi0, gi1, gch, gsz, perm = dec_core[c]
        di0 = np.concatenate([np.pad(gi0[g], (0, (dgc[g] - gch[g]) * P))
                              for g in range(4)])
        di1 = np.concatenate([np.pad(gi1[g], (0, (dgc[g] - gch[g]) * P))
                              for g in range(4)])
        pm = np.concatenate([np.pad(perm[g], (0, (dgc[g] - gch[g]) * P),
                                    constant_values=-1) for g in range(4)])
        out_perms.append(pm)

        # per-call exact counts (this core); guard all-pad calls with one
        # valid idx 0 so the ucode never sees num_idxs_reg == 0
        cnt1 = meta["cnt1"]
        cnt2 = meta["cnt2"]
        ch1_off = meta["ch1_off"]
        ch2_off = meta["ch2_off"]
        dgoff = np.concatenate([[0], np.cumsum(dgc)])[:-1]
        gcv = []
        for kind, b_, hh_, s_, cw_ in _call_plan(meta):
            if kind == "l1":
                real = int(min(max(cnt1[c, b_] - s_ * P, 0), cw_ * P))
                pos = (int(ch1_off[b_]) + s_) * P
                arr = idx1
            elif kind == "l2":
                real = int(min(max(cnt2[c, b_, hh_] - s_ * P, 0), cw_ * P))
                pos = (int(ch2_off[b_, hh_]) + s_) * P
                arr = idx2
            elif kind == "d0":
                real = int(min(max(gsz[b_] - s_ * P, 0), cw_ * P))
                pos = (int(dgoff[b_]) + s_) * P
                arr = di0
            else:
                real = int(min(max(gsz[b_] - s_ * P, 0), cw_ * P))
                pos = (int(dgoff[b_]) + s_) * P
                arr = di1
            if real == 0:
                real = 1
            gcv.append(real)
        gc = np.tile(np.asarray(gcv, np.int32), (P, 1))

        m = dict(common)
        m["gc"] = gc
        m["idx1"] = _wrap16(idx1.astype(np.int16))
        m["idx2"] = _wrap16(idx2.astype(np.int16))
        m["xn"] = _wrap16(xn_all[c].astype(np.int16))
        m["oh1"] = oh1.reshape(P, TOTE1).view(ml_dtypes.float8_e4m3)
        m["oht1"] = oht1.reshape(P, TOTE1).view(ml_dtypes.float8_e4m3)
        m["oh2"] = oh2.reshape(P, TOTE2).view(ml_dtypes.float8_e4m3)
        m["oht2"] = oht2.reshape(P, TOTE2).view(ml_dtypes.float8_e4m3)
        m["di0"] = _wrap16(di0.astype(np.int16))
        m["di1"] = _wrap16(di1.astype(np.int16))
        in_maps.append(m)

    return in_maps, out_perms, meta


def kernel(**inputs):
    in_maps, out_perms, meta = _make_inputs(inputs)
    nc = _build_nc(meta)
    import os
    trace = bool(int(os.environ.get("GAT_TRACE", "0")))
    if trace:
        try:
            import sys as _sys, types as _types
            import antenv as _antenv
            from trn_agent_boot.trn_boot import _ntff_profile_via_ctypes as _np_hook
            _hm = _types.ModuleType("antenv.axon_hooks")
            _hm.get_axon_ntff_profile_hook = (
                lambda: _np_hook('/opt/axon/libaxon_pjrt.so'))
            _hm.set_axon_ntff_profile_hook = lambda h: None
            _sys.modules["antenv.axon_hooks"] = _hm
            _antenv.axon_hooks = _hm
        except Exception:
            trace = False
    r = run_bass_kernel_spmd(nc, in_maps, core_ids=list(range(NC)), trace=trace)
    if trace and r.exec_time_ns:
        print("HW exec time: %d ns" % r.exec_time_ns)
        if r.per_core_scope_times:
            for s, mm in sorted(r.per_core_scope_times.items()):
                print("  scope %-8s %s" % (s, {k: "%dus" % (v // 1000) for k, v in mm.items()}))
        if r.instructions_and_trace:
            print("trace:", r.instructions_and_trace[1])

    out = np.zeros(EL, np.float32)
    for c in range(NC):
        resv = r.results[c]["out"]
        pm = out_perms[c]
        vals = resv.T.reshape(-1)
        valid = pm >= 0
        out[c * ELC + pm[valid]] = vals[valid]
    return out


if __name__ == "__main__":
    d = np.load("/root/problem/ref_data.npz")
    inputs = {k: d[k] for k in ("x", "edge_index", "edge_label_index", "emb",
                                "W1", "att_src1", "att_dst1", "b1",
                                "W2", "att_src2", "att_dst2", "b2")}
    got = kernel(**inputs)
    exp = d["expected"]
    denom = np.abs(exp).mean()
    rel = np.abs(got - exp) / denom
    print("Relative error: max %.3e mean %.3e" % (rel.max(), rel.mean()))


# revision 29
# speedup vs baseline: 1.1050x; 1.1050x over previous
"""GAT link-prediction kernel for 8 Trainium2 NeuronCores (Bass/Tile).

v2 design (vs baseline):
- Layer 2 reformulated: W2 projection commutes with the attention-weighted
  scatter-sum (xl2 = h1 @ W2 is linear), so the per-edge gather pulls 256B
  rows [h1(32)|a_src2~(8)|pad] instead of 768B projected rows; W2 applied
  per dst block after softmax normalization via two K=128 matmuls on the
  transposed accumulator.
- Layer-2 table is built as a per-core 1.6MB shard and replicated by one
  AllGather (replaces the 620us redundant build2 phase).
- Layer 1 gathers 512B xl-only rows; per-edge a_src1 recomputed on DVE
  (mult + reduce against att_src1).
- Per-layer edge bucketing: L1 needs no src-half split (vocab idx < 5120);
  L2 buckets split by SOURCE SLOT RANGE (A=slots 0-24, B=25-48) so the
  A-half table AllGather overlaps layer-1 blocks 25-48; only the B-half
  AG (~50us) is exposed, and L2's A-bucket gathers start before AG-B ends.
- Dst blocks rebalanced across cores (greedy by edge count) to cut padding.
- z packed to 32 f32/row; decode gathers use 256B elements with a parity
  byte-offset trick (even rows from base+0, odd from base+128B).
"""

import numpy as np
import ml_dtypes

import concourse.bass as bass
import concourse.bacc as bacc
import concourse.mybir as mybir
import concourse.tile as tile
from concourse.bass_utils import run_bass_kernel_spmd
from concourse.masks import make_identity

P = 128
NC = 8
N = 50000
V = 5000
VPAD = 5120
EL = 200000
D = 128
HID = 32
OUT = 32
H = 8
NEG = 0.2
NSLOT = 49
NBLK = NC * NSLOT          # 392
RPC = NSLOT * P            # 6272 table rows per core
NROWS = NC * RPC           # 50176
HALFROW = NROWS // 2       # 25088
ELC = EL // NC
F32 = mybir.dt.float32
BF16 = mybir.dt.bfloat16
FP8 = mybir.dt.float8e4
I16 = mybir.dt.int16
EXP = mybir.ActivationFunctionType.Exp
RELU = mybir.ActivationFunctionType.Relu
MULT = mybir.AluOpType.mult
ADD = mybir.AluOpType.add
MAXOP = mybir.AluOpType.max
AXX = mybir.AxisListType.X


def _wrap16(idx_flat):
    n = len(idx_flat)
    assert n % 16 == 0
    w = np.zeros((16, n // 16), np.int16)
    w[np.arange(n) % 16, np.arange(n) // 16] = idx_flat
    return np.tile(w, (8, 1))


def _plan(x, edge_index, eli):
    xs = x.astype(np.int64)
    src = np.concatenate([edge_index[0], np.arange(N)]).astype(np.int64)
    dst = np.concatenate([edge_index[1], np.arange(N)]).astype(np.int64)
    eblk = dst // P
    cntb = np.bincount(eblk, minlength=NBLK)

    # greedy block -> (core, slot) assignment balancing edge counts
    order = np.argsort(-cntb, kind="stable")
    core_of = np.zeros(NBLK, np.int64)
    slot_of = np.zeros(NBLK, np.int64)
    load = np.zeros(NC, np.int64)
    for s in range(NSLOT):
        grp = order[s * 8:(s + 1) * 8]
        gg = grp[np.argsort(-cntb[grp], kind="stable")]
        cores = np.argsort(load, kind="stable")
        for k, b in enumerate(gg):
            core_of[b] = cores[k]
            slot_of[b] = s
            load[cores[k]] += cntb[b]

    # pos-major row order: matches both the [p, (slot e)] shard write and the
    # z AllGather layout (core, pos, slot)
    nodes = np.arange(N)
    tabrow = (core_of[nodes // P] * RPC + (nodes % P) * NSLOT
              + slot_of[nodes // P])

    ecore = core_of[eblk]
    eslot = slot_of[eblk]
    dpos = dst % P

    # ---- layer 1 buckets: (core, slot), sorted by vocab id ----
    o1 = np.lexsort((xs[src], eslot, ecore))
    cnt1 = np.zeros((NC, NSLOT), np.int64)
    np.add.at(cnt1, (ecore, eslot), 1)
    CH1 = np.maximum(1, -(-cnt1.max(axis=0) // P))
    ch1_off = np.concatenate([[0], np.cumsum(CH1)])[:-1]
    TOTCH1 = int(CH1.sum())
    TOTE1 = TOTCH1 * P

    # ---- layer 2 buckets: (core, slot, A/B) split by SOURCE slot range ----
    # A = src slots 0-24 (table ready after L1 block 24), B = slots 25-48
    c_src = slot_of[src // P]
    p_src = src % P
    r_src = core_of[src // P]
    half = (c_src >= 25).astype(np.int64)
    trow2 = np.where(half == 0,
                     r_src * 3200 + p_src * 25 + c_src,
                     r_src * 3072 + p_src * 24 + (c_src - 25))
    o2 = np.lexsort((trow2, half, eslot, ecore))
    cnt2 = np.zeros((NC, NSLOT, 2), np.int64)
    np.add.at(cnt2, (ecore, eslot, half), 1)
    CH2 = -(-cnt2.max(axis=0) // P)
    ch2_off = np.zeros((NSLOT, 2), np.int64)
    run = 0
    for s in range(NSLOT):
        ch2_off[s, 0] = run
        run += CH2[s, 0]
        ch2_off[s, 1] = run
        run += CH2[s, 1]
    TOTCH2 = int(run)
    TOTE2 = TOTCH2 * P

    # flat start offsets of each core's buckets in the sorted order
    per_core = []
    start1 = np.zeros((NC, NSLOT), np.int64)
    pos = 0
    for c in range(NC):
        for s in range(NSLOT):
            start1[c, s] = pos
            pos += cnt1[c, s]
    start2 = np.zeros((NC, NSLOT, 2), np.int64)
    pos = 0
    for c in range(NC):
        for s in range(NSLOT):
            for h in range(2):
                start2[c, s, h] = pos
                pos += cnt2[c, s, h]

    src1 = src[o1]
    dst1p = dpos[o1]
    trow2s = trow2[o2]
    dst2p = dpos[o2]
    for c in range(NC):
        idx1 = np.zeros(TOTE1, np.int64)
        dl1 = np.full(TOTE1, -1, np.int64)
        for s in range(NSLOT):
            nr = int(cnt1[c, s])
            s0 = int(start1[c, s])
            o0 = int(ch1_off[s]) * P
            idx1[o0:o0 + nr] = xs[src1[s0:s0 + nr]]
            dl1[o0:o0 + nr] = dst1p[s0:s0 + nr]
        idx2 = np.zeros(TOTE2, np.int64)
        dl2 = np.full(TOTE2, -1, np.int64)
        for s in range(NSLOT):
            for h in range(2):
                nr = int(cnt2[c, s, h])
                s0 = int(start2[c, s, h])
                o0 = int(ch2_off[s, h]) * P
                idx2[o0:o0 + nr] = trow2s[s0:s0 + nr]
                dl2[o0:o0 + nr] = dst2p[s0:s0 + nr]
        per_core.append((idx1, dl1, idx2, dl2))

    # ---- decode plan: 4 parity groups ----
    z0 = tabrow[eli[0]]
    z1 = tabrow[eli[1]]
    dec_grp_chunks = np.zeros(4, np.int64)
    dec_core = []
    for c in range(NC):
        a = z0[c * ELC:(c + 1) * ELC]
        b = z1[c * ELC:(c + 1) * ELC]
        grp = (a & 1) * 2 + (b & 1)
        order_d = np.argsort(grp, kind="stable")
        gi0, gi1, gch, gsz, perm = [], [], [], [], []
        for g in range(4):
            m = grp[order_d] == g
            ids0 = (a[order_d][m]) >> 1
            ids1 = (b[order_d][m]) >> 1
            pidx = order_d[m]
            gsz.append(len(ids0))
            npad = (-len(ids0)) % P
            ids0 = np.concatenate([ids0, np.zeros(npad, np.int64)])
            ids1 = np.concatenate([ids1, np.zeros(npad, np.int64)])
            pidx = np.concatenate([pidx, np.full(npad, -1)])
            gch.append(len(ids0) // P)
            gi0.append(ids0)
            gi1.append(ids1)
            perm.append(pidx)
        dec_grp_chunks = np.maximum(dec_grp_chunks, gch)
        dec_core.append((gi0, gi1, gch, gsz, perm))

    meta = dict(CH1=CH1, ch1_off=ch1_off, TOTCH1=TOTCH1, TOTE1=TOTE1,
                CH2=CH2, ch2_off=ch2_off, TOTCH2=TOTCH2, TOTE2=TOTE2,
                dec_grp_chunks=[int(v) for v in dec_grp_chunks],
                core_of=core_of, slot_of=slot_of, tabrow=tabrow,
                cnt1=cnt1, cnt2=cnt2)
    return per_core, dec_core, meta


def _call_plan(meta):
    """Gather-call order shared by device program and host count tables.
    Entries: (kind, slot, half, chunk_start, n_chunks)."""
    CH1 = meta["CH1"]
    CH2 = meta["CH2"]
    dgc = meta["dec_grp_chunks"]
    calls = []
    for b in range(NSLOT):
        C = int(CH1[b])
        for s in range(0, C, 8):
            calls.append(("l1", b, 0, s, min(8, C - s)))
    for b in range(NSLOT):
        for hh in (0, 1):
            for s in range(0, int(CH2[b, hh]), 8):
                calls.append(("l2", b, hh, s, min(8, int(CH2[b, hh]) - s)))
    for gi in range(4):
        for s in range(0, dgc[gi], 8):
            calls.append(("d0", gi, 0, s, min(8, dgc[gi] - s)))
            calls.append(("d1", gi, 1, s, min(8, dgc[gi] - s)))
    return calls


def _build_nc(meta):
    CH1 = meta["CH1"]
    ch1_off = meta["ch1_off"]
    TOTE1 = meta["TOTE1"]
    CH2 = meta["CH2"]
    ch2_off = meta["ch2_off"]
    TOTE2 = meta["TOTE2"]
    dgc = meta["dec_grp_chunks"]
    DGC = sum(dgc)
    CM1 = int(CH1.max())
    CM2 = int(CH2.sum(axis=1).max())
    CMX = max(CM1, CM2)

    nc = bacc.Bacc("TRN2", target_bir_lowering=False, debug=False,
                   num_devices=NC, num_swdge_queues=4)

    t_embT = nc.dram_tensor("embT", [D, VPAD], BF16, kind="ExternalInput")
    t_w1x = nc.dram_tensor("w1x", [D, 272], BF16, kind="ExternalInput")
    t_b1 = nc.dram_tensor("b1", [P, HID], F32, kind="ExternalInput")
    t_b2 = nc.dram_tensor("b2", [P, OUT], F32, kind="ExternalInput")
    t_w2s = nc.dram_tensor("w2s", [HID, 8], BF16, kind="ExternalInput")
    t_w2d = nc.dram_tensor("w2d", [HID, 8], BF16, kind="ExternalInput")
    t_w2stk = nc.dram_tensor("w2stk", [P, 64], F32, kind="ExternalInput")
    t_idx1 = nc.dram_tensor("idx1", [P, TOTE1 // 16], I16, kind="ExternalInput")
    t_idx2 = nc.dram_tensor("idx2", [P, TOTE2 // 16], I16, kind="ExternalInput")
    t_xn = nc.dram_tensor("xn", [P, RPC // 16], I16, kind="ExternalInput")
    t_oh1 = nc.dram_tensor("oh1", [P, TOTE1], FP8, kind="ExternalInput")
    t_oht1 = nc.dram_tensor("oht1", [P, TOTE1], FP8, kind="ExternalInput")
    t_oh2 = nc.dram_tensor("oh2", [P, TOTE2], FP8, kind="ExternalInput")
    t_oht2 = nc.dram_tensor("oht2", [P, TOTE2], FP8, kind="ExternalInput")
    calls = _call_plan(meta)
    NCALL = len(calls)
    t_gc = nc.dram_tensor("gc", [P, NCALL], mybir.dt.int32, kind="ExternalInput")
    t_di0 = nc.dram_tensor("di0", [P, DGC * 8], I16, kind="ExternalInput")
    t_di1 = nc.dram_tensor("di1", [P, DGC * 8], I16, kind="ExternalInput")
    t_out = nc.dram_tensor("out", [P, DGC], F32, kind="ExternalOutput")

    rr = [0]

    def nextq():
        rr[0] = (rr[0] + 1) % 4
        return rr[0]

    with tile.TileContext(nc) as tc:
        with (
            tc.tile_pool(name="dram", bufs=1, space="DRAM") as dp,
            tc.tile_pool(name="const", bufs=1) as cp,
            tc.tile_pool(name="sm", bufs=6) as sm,
            tc.tile_pool(name="psA", bufs=3, space="PSUM") as psA,
            tc.tile_pool(name="psB", bufs=2, space="PSUM") as psB,
            tc.tile_pool(name="psC", bufs=1, space="PSUM") as psC,
            tc.tile_pool(name="psD", bufs=2, space="PSUM") as psD,
        ):
            tab1 = dp.tile([VPAD, 384], BF16)
            tab2shA = dp.tile([3200, P], BF16)
            tab2shB = dp.tile([3072, P], BF16)
            tabAfull = dp.tile([NC * 3200, P], BF16, addr_space="Shared")
            tabBfull = dp.tile([NC * 3072, P], BF16, addr_space="Shared")
            zloc = dp.tile([P, NSLOT * 32], F32)
            zall = dp.tile([1026, NSLOT * 32], F32, addr_space="Shared")

            # ---------- constants ----------
            ident = cp.tile([P, P], F32)
            make_identity(nc, ident[:])
            identb = cp.tile([P, P], BF16)
            nc.vector.tensor_copy(out=identb[:], in_=ident[:])
            embT_sb = cp.tile([D, VPAD], BF16)
            nc.sync.dma_start(out=embT_sb[:], in_=t_embT[:, :])
            w1x_sb = cp.tile([D, 272], BF16)
            nc.sync.dma_start(out=w1x_sb[:], in_=t_w1x[:, :])
            b1_sb = cp.tile([P, HID], F32)
            nc.sync.dma_start(out=b1_sb[:], in_=t_b1[:, :])
            b2_sb = cp.tile([P, OUT], F32)
            nc.sync.dma_start(out=b2_sb[:], in_=t_b2[:, :])
            w2s_sb = cp.tile([HID, 8], BF16)
            nc.sync.dma_start(out=w2s_sb[:], in_=t_w2s[:, :])
            w2d_sb = cp.tile([HID, 8], BF16)
            nc.sync.dma_start(out=w2d_sb[:], in_=t_w2d[:, :])
            w2stk_sb = cp.tile([P, 64], F32)
            nc.sync.dma_start(out=w2stk_sb[:], in_=t_w2stk[:, :])
            xn_sb = cp.tile([P, RPC // 16], I16)
            nc.sync.dma_start(out=xn_sb[:], in_=t_xn[:, :])
            an1_sb = cp.tile([P, NSLOT, 8], BF16)
            an2_sb = cp.tile([P, NSLOT, 8], BF16)
            h1t_sb = cp.tile([HID, RPC], BF16)
            t2row_sb = cp.tile([P, NSLOT, P], BF16)
            z_sb = cp.tile([P, NSLOT, 32], F32)
            gc_sb = cp.tile([P, NCALL], mybir.dt.int32)
            nc.sync.dma_start(out=gc_sb[:], in_=t_gc[:, :])
            greg = nc.gpsimd.alloc_register("gcnt")
            kk = [0]

            def gather_exact(out_ap, in_ap, idxs_ap, n_static, elem):
                kk[0] += 1
                nc.gpsimd.dma_gather(out_ap, in_ap, idxs_ap, n_static,
                                     n_static, elem, queue_num=nextq())

            di0_sb = cp.tile([P, DGC * 8], I16)
            nc.sync.dma_start(out=di0_sb[:], in_=t_di0[:, :])
            di1_sb = cp.tile([P, DGC * 8], I16)
            nc.sync.dma_start(out=di1_sb[:], in_=t_di1[:, :])
            res = cp.tile([P, DGC], F32)

            nc.vector.memset(t2row_sb[:, :, 40:P], 0)

            # ---------- build tab1: xl1 = emb @ W1, 512B rows ----------
            with nc.named_scope("build1"):
                with (
                    tc.tile_pool(name="b1p", bufs=3) as bp,
                ):
                    for tv in range(VPAD // P):
                        acc = psB.tile([P, 288], F32, space="PSUM", tag="big")
                        nc.tensor.matmul(out=acc[:, 0:272],
                                         lhsT=embT_sb[:, tv * P:(tv + 1) * P],
                                         rhs=w1x_sb[:], start=True, stop=True)
                        ob = bp.tile([P, 272], BF16, tag="obf")
                        nc.scalar.copy(out=ob[:], in_=acc[:, 0:272])
                        nc.sync.dma_start(out=tab1[tv * P:(tv + 1) * P, 0:272],
                                          in_=ob[:])

                    # an1: per-node a_dst1 from the table's att_dst column
                    for c0 in range(0, NSLOT, 8):
                        cw = min(8, NSLOT - c0)
                        gt = bp.tile([P, 8, 384], BF16, tag="ang")
                        nc.gpsimd.dma_gather(gt[:, 0:cw, :], tab1[:, :],
                                             xn_sb[:, c0 * 8:(c0 + cw) * 8],
                                             cw * P, cw * P, 384,
                                             queue_num=nextq())
                        nc.vector.tensor_copy(out=an1_sb[:, c0:c0 + cw, :],
                                              in_=gt[:, 0:cw, 264:272])

            # ---------- layer 1 ----------
            with nc.named_scope("layer1"):
                with (
                    tc.tile_pool(name="l1g", bufs=3) as gp,
                ):
                    for _ in range(4):
                        gz = gp.tile([P, CM1, 384], BF16, tag="g1", bufs=4)
                        nc.vector.memset(gz[:], 0)
                    for b in range(NSLOT):
                        C = int(CH1[b])
                        base = int(ch1_off[b])
                        idxb = sm.tile([P, CM1 * 8], I16, tag="idxb")
                        nc.sync.dma_start(out=idxb[:, 0:C * 8],
                                          in_=t_idx1[:, base * 8:(base + C) * 8])
                        g1 = gp.tile([P, CM1, 384], BF16, tag="g1", bufs=4)
                        for s in range(0, C, 8):
                            cw = min(8, C - s)
                            gather_exact(g1[:, s:s + cw, :], tab1[:, :],
                                         idxb[:, s * 8:(s + cw) * 8],
                                         cw * P, 384)
                        oh = gp.tile([P, CM1 * P], FP8, tag="oh", bufs=4)
                        nc.sync.dma_start(out=oh[:, 0:C * P],
                                          in_=t_oh1[:, base * P:(base + C) * P])
                        oht = gp.tile([P, CM1 * P], FP8, tag="oht", bufs=4)
                        nc.sync.dma_start(out=oht[:, 0:C * P],
                                          in_=t_oht1[:, base * P:(base + C) * P])
                        # a_dst per edge via transposed one-hot
                        adp = psC.tile([P, CMX * 8], F32, space="PSUM", tag="adp")
                        for j in range(C):
                            nc.tensor.matmul(out=adp[:, j * 8:(j + 1) * 8],
                                             lhsT=oht[:, j * P:(j + 1) * P],
                                             rhs=an1_sb[:, b, :],
                                             start=True, stop=True)
                        te = sm.tile([P, CM1, 8], F32, tag="te")
                        nc.vector.tensor_tensor(
                            out=te[:, 0:C, :], in0=g1[:, 0:C, 256:264],
                            in1=adp[:, 0:C * 8].rearrange("p (c a) -> p c a", a=8),
                            op=ADD)
                        e1 = sm.tile([P, CM1, 8], F32, tag="e1")
                        nc.scalar.activation(out=e1[:, 0:C, :], in_=te[:, 0:C, :],
                                             func=EXP)
                        e2 = sm.tile([P, CM1, 8], F32, tag="e2")
                        nc.scalar.activation(out=e2[:, 0:C, :], in_=te[:, 0:C, :],
                                             func=EXP, scale=NEG)
                        ee = sm.tile([P, CM1, 8], BF16, tag="ee")
                        nc.vector.tensor_tensor(out=ee[:, 0:C, :],
                                                in0=e1[:, 0:C, :],
                                                in1=e2[:, 0:C, :], op=MAXOP)
                        # weighted messages in place; ee into cols 256:264
                        eex = gp.tile([P, CM1, 256], BF16, tag="eex", bufs=2)
                        nc.scalar.copy(
                            out=eex[:, 0:C, :].rearrange(
                                "p c (h o) -> p c h o", h=H),
                            in_=ee[:, 0:C, :].to_broadcast([P, C, 8, HID]))
                        nc.vector.tensor_tensor(
                            out=g1[:, 0:C, 0:256], in0=g1[:, 0:C, 0:256],
                            in1=eex[:, 0:C, :], op=MULT)
                        nc.vector.tensor_copy(out=g1[:, 0:C, 256:264],
                                              in_=ee[:, 0:C, :])
                        acc = psA.tile([P, 264], F32, space="PSUM", tag="acc")
                        for j in range(C):
                            nc.tensor.matmul(out=acc[:],
                                             lhsT=oh[:, j * P:(j + 1) * P],
                                             rhs=g1[:, j, 0:264],
                                             start=(j == 0), stop=(j == C - 1))
                        rec = sm.tile([P, 8], F32, tag="rec")
                        nc.vector.tensor_scalar(out=rec[:], in0=acc[:, 256:264],
                                                scalar1=8.0, scalar2=1e-30,
                                                op0=MULT, op1=ADD)
                        nc.vector.reciprocal(out=rec[:], in_=rec[:])
                        hs = sm.tile([P, 256], F32, tag="hs")
                        nc.vector.tensor_tensor(
                            out=hs[:].rearrange("p (h o) -> p h o", h=H),
                            in0=acc[:, 0:256].rearrange("p (h o) -> p h o", h=H),
                            in1=rec[:].to_broadcast([P, H, HID]), op=MULT)
                        hsum = sm.tile([P, 32], F32, tag="hsum")
                        nc.vector.tensor_reduce(
                            out=hsum[:],
                            in_=hs[:].rearrange("p (h o) -> p o h", h=H),
                            axis=AXX, op=ADD)
                        nc.vector.tensor_add(out=hsum[:], in0=hsum[:],
                                             in1=b1_sb[:])
                        nc.scalar.activation(out=hsum[:], in_=hsum[:],
                                             func=RELU)
                        # h1 row (bf16) into the tab2 shard staging tile
                        nc.scalar.copy(out=t2row_sb[:, b, 0:32],
                                       in_=hsum[:])
                        # transposed h1 for the per-node attention projections
                        tp = psB.tile([P, 288], F32, space="PSUM", tag="big")
                        nc.tensor.transpose(out=tp[0:HID, 0:P],
                                            in_=hsum[:],
                                            identity=ident[:])
                        nc.scalar.copy(out=h1t_sb[:, b * P:(b + 1) * P],
                                       in_=tp[0:HID, 0:P])
                        a2p = psD.tile([P, 32], F32, space="PSUM", tag="small")
                        nc.tensor.matmul(out=a2p[:, 0:8],
                                         lhsT=h1t_sb[:, b * P:(b + 1) * P],
                                         rhs=w2s_sb[:], start=True, stop=True)
                        nc.tensor.matmul(out=a2p[:, 8:16],
                                         lhsT=h1t_sb[:, b * P:(b + 1) * P],
                                         rhs=w2d_sb[:], start=True, stop=True)
                        nc.scalar.copy(out=t2row_sb[:, b, 32:40],
                                        in_=a2p[:, 0:8])
                        nc.scalar.copy(out=an2_sb[:, b, :],
                                        in_=a2p[:, 8:16])
                        if b == 24:
                            # A-half table ready: ship + allgather it while
                            # layer 1 keeps running on slots 25-48
                            nc.sync.dma_start(
                                out=tab2shA[:, :].rearrange(
                                    "(p c) e -> p (c e)", p=P),
                                in_=t2row_sb[:, 0:25, :].rearrange(
                                    "p c e -> p (c e)"))
                            nc.gpsimd.collective_compute(
                                "AllGather", mybir.AluOpType.bypass,
                                replica_groups=[list(range(NC))],
                                ins=[tab2shA[:, :].opt()],
                                outs=[tabAfull[:, :].opt()])
                    nc.sync.dma_start(
                        out=tab2shB[:, :].rearrange("(p c) e -> p (c e)", p=P),
                        in_=t2row_sb[:, 25:NSLOT, :].rearrange("p c e -> p (c e)"))

            # ---------- allgather B-half ----------
            nc.gpsimd.collective_compute(
                "AllGather", mybir.AluOpType.bypass,
                replica_groups=[list(range(NC))],
                ins=[tab2shB[:, :].opt()], outs=[tabBfull[:, :].opt()])

            # ---------- layer 2 ----------
            with nc.named_scope("layer2"):
                with (
                    tc.tile_pool(name="l2g", bufs=4) as gp2,
                    tc.tile_pool(name="l2r", bufs=3) as rp2,
                ):
                    for _ in range(4):
                        gz2 = gp2.tile([P, CM2, P], BF16, tag="g2")
                        nc.vector.memset(gz2[:], 0)
                    tabs = (tabAfull[:, :], tabBfull[:, :])
                    for b in range(NSLOT):
                        c0h = [int(ch2_off[b, 0]), int(ch2_off[b, 1])]
                        cws = [int(CH2[b, 0]), int(CH2[b, 1])]
                        C = cws[0] + cws[1]
                        base = c0h[0]
                        idxb = sm.tile([P, CM2 * 8], I16, tag="idxb2")
                        nc.sync.dma_start(out=idxb[:, 0:C * 8],
                                          in_=t_idx2[:, base * 8:(base + C) * 8])
                        g2 = gp2.tile([P, CM2, P], BF16, tag="g2")
                        for hh in (0, 1):
                            off = c0h[hh] - base
                            for s in range(0, cws[hh], 8):
                                cw = min(8, cws[hh] - s)
                                gather_exact(
                                    g2[:, off + s:off + s + cw, :], tabs[hh],
                                    idxb[:, (off + s) * 8:(off + s + cw) * 8],
                                    cw * P, P)
                        oh = gp2.tile([P, CM2 * P], FP8, tag="oh2")
                        nc.sync.dma_start(out=oh[:, 0:C * P],
                                          in_=t_oh2[:, base * P:(base + C) * P])
                        oht = gp2.tile([P, CM2 * P], FP8, tag="oht2")
                        nc.sync.dma_start(out=oht[:, 0:C * P],
                                          in_=t_oht2[:, base * P:(base + C) * P])
                        adp = psC.tile([P, CMX * 8], F32, space="PSUM", tag="adp")
                        for j in range(C):
                            nc.tensor.matmul(out=adp[:, j * 8:(j + 1) * 8],
                                             lhsT=oht[:, j * P:(j + 1) * P],
                                             rhs=an2_sb[:, b, :],
                                             start=True, stop=True)
                        te = sm.tile([P, CM2, 8], F32, tag="te2")
                        nc.vector.tensor_tensor(
                            out=te[:, 0:C, :], in0=g2[:, 0:C, 32:40],
                            in1=adp[:, 0:C * 8].rearrange("p (c a) -> p c a", a=8),
                            op=ADD)
                        e1 = sm.tile([P, CM2, 8], F32, tag="e12")
                        nc.scalar.activation(out=e1[:, 0:C, :], in_=te[:, 0:C, :],
                                             func=EXP)
                        e2 = sm.tile([P, CM2, 8], F32, tag="e22")
                        nc.scalar.activation(out=e2[:, 0:C, :], in_=te[:, 0:C, :],
                                             func=EXP, scale=NEG)
                        ee = sm.tile([P, CM2, 8], BF16, tag="ee2")
                        nc.vector.tensor_tensor(out=ee[:, 0:C, :],
                                                in0=e1[:, 0:C, :],
                                                in1=e2[:, 0:C, :], op=MAXOP)
                        # rhs: outer product ee[h] x h1[c] per edge
                        r2 = rp2.tile([P, CM2, 264], BF16, tag="r2")
                        eex2 = rp2.tile([P, CM2, 256], BF16, tag="eex2")
                        nc.scalar.copy(
                            out=eex2[:, 0:C, :].rearrange(
                                "p c (h o) -> p c h o", h=H),
                            in_=ee[:, 0:C, :].to_broadcast([P, C, 8, HID]))
                        nc.vector.tensor_tensor(
                            out=r2[:, 0:C, 0:256].rearrange(
                                "p c (h o) -> p c h o", h=H),
                            in0=eex2[:, 0:C, :].rearrange(
                                "p c (h o) -> p c h o", h=H),
                            in1=g2[:, 0:C, None, 0:32].broadcast_to(
                                [P, C, H, HID]),
                            op=MULT)
                        nc.vector.tensor_copy(out=r2[:, 0:C, 256:264],
                                              in_=ee[:, 0:C, :])
                        acc = psA.tile([P, 264], F32, space="PSUM", tag="acc")
                        for j in range(C):
                            nc.tensor.matmul(out=acc[:],
                                             lhsT=oh[:, j * P:(j + 1) * P],
                                             rhs=r2[:, j, :],
                                             start=(j == 0), stop=(j == C - 1))
                        rec = sm.tile([P, 8], F32, tag="rec2")
                        nc.vector.tensor_scalar(out=rec[:], in0=acc[:, 256:264],
                                                scalar1=1.0, scalar2=1e-30,
                                                op0=MULT, op1=ADD)
                        nc.vector.reciprocal(out=rec[:], in_=rec[:])
                        hsn = sm.tile([P, 256], F32, tag="hsn")
                        nc.vector.tensor_tensor(
                            out=hsn[:].rearrange("p (h o) -> p h o", h=H),
                            in0=acc[:, 0:256].rearrange("p (h o) -> p h o", h=H),
                            in1=rec[:].to_broadcast([P, H, HID]), op=MULT)
                        # transpose both halves, project with W2 (1/8 folded)
                        s2t = sm.tile([P, 256], F32, tag="s2t")
                        for k in range(2):
                            tp2 = psB.tile([P, 288], F32, space="PSUM", tag="big")
                            nc.tensor.transpose(out=tp2[:, 0:P],
                                                in_=hsn[:, k * P:(k + 1) * P],
                                                identity=ident[:])
                            nc.scalar.copy(out=s2t[:, k * P:(k + 1) * P],
                                           in_=tp2[:, 0:P])
                        zmm = psD.tile([P, 32], F32, space="PSUM", tag="small")
                        nc.tensor.matmul(out=zmm[:], lhsT=s2t[:, 0:P],
                                         rhs=w2stk_sb[:, 0:32],
                                         start=True, stop=False)
                        nc.tensor.matmul(out=zmm[:], lhsT=s2t[:, P:256],
                                         rhs=w2stk_sb[:, 32:64],
                                         start=False, stop=True)
                        nc.vector.tensor_add(out=z_sb[:, b, :], in0=zmm[:],
                                             in1=b2_sb[:])
                    nc.sync.dma_start(out=zloc[:, :],
                                      in_=z_sb[:].rearrange("p c a -> p (c a)"))

            # ---------- allgather z ----------
            nc.gpsimd.collective_compute(
                "AllGather", mybir.AluOpType.bypass,
                replica_groups=[list(range(NC))],
                ins=[zloc[:, :].opt()], outs=[zall[0:NC * P, :].opt()])

            # ---------- decode ----------
            with nc.named_scope("decode"):
                with tc.tile_pool(name="dg", bufs=4) as dgp:
                    zflat = zall[:, :].rearrange("p f -> (p f)")
                    NPAIR = (1026 * NSLOT * 32) // 64
                    zeven = zflat.rearrange("(r e) -> r e", e=64)
                    zodd = zflat[32:32 + (NPAIR - 1) * 64].rearrange(
                        "(r e) -> r e", e=64)
                    goff = 0
                    for gi in range(4):
                        gch = dgc[gi]
                        v0 = zeven if (gi >> 1) == 0 else zodd
                        v1 = zeven if (gi & 1) == 0 else zodd
                        for s in range(0, gch, 8):
                            cw = min(8, gch - s)
                            z0t = dgp.tile([P, 8, 64], F32, tag="z0")
                            gather_exact(
                                z0t[:, 0:cw, :], v0,
                                di0_sb[:, (goff + s) * 8:(goff + s + cw) * 8],
                                cw * P, 64)
                            z1t = dgp.tile([P, 8, 64], F32, tag="z1")
                            gather_exact(
                                z1t[:, 0:cw, :], v1,
                                di1_sb[:, (goff + s) * 8:(goff + s + cw) * 8],
                                cw * P, 64)
                            nc.vector.tensor_tensor(out=z0t[:, 0:cw, 0:32],
                                                    in0=z0t[:, 0:cw, 0:32],
                                                    in1=z1t[:, 0:cw, 0:32],
                                                    op=MULT)
                            nc.vector.tensor_reduce(
                                out=res[:, goff + s:goff + s + cw],
                                in_=z0t[:, 0:cw, 0:32], axis=AXX, op=ADD)
                        goff += gch
                    nc.sync.dma_start(out=t_out[:, :], in_=res[:])

    nc.compile()
    return nc


def _make_inputs(inputs):
    x = np.asarray(inputs["x"]).astype(np.int64)
    edge_index = np.asarray(inputs["edge_index"]).astype(np.int64)
    eli = np.asarray(inputs["edge_label_index"]).astype(np.int64)
    emb = np.asarray(inputs["emb"]).astype(np.float32)
    W1 = np.asarray(inputs["W1"]).astype(np.float32)
    W2 = np.asarray(inputs["W2"]).astype(np.float32)
    a1s = np.asarray(inputs["att_src1"]).astype(np.float32)
    a1d = np.asarray(inputs["att_dst1"]).astype(np.float32)
    a2s = np.asarray(inputs["att_src2"]).astype(np.float32)
    a2d = np.asarray(inputs["att_dst2"]).astype(np.float32)
    b1 = np.asarray(inputs["b1"]).astype(np.float32).reshape(-1)
    b2 = np.asarray(inputs["b2"]).astype(np.float32).reshape(-1)

    per_core, dec_core, meta = _plan(x, edge_index, eli)
    TOTCH1 = meta["TOTCH1"]
    TOTE1 = meta["TOTE1"]
    TOTCH2 = meta["TOTCH2"]
    TOTE2 = meta["TOTE2"]
    dgc = meta["dec_grp_chunks"]
    DGC = sum(dgc)
    core_of = meta["core_of"]
    slot_of = meta["slot_of"]

    bf = ml_dtypes.bfloat16
    emb_pad = np.zeros((VPAD, D), np.float32)
    emb_pad[:V] = emb
    # W~2s[d,h] = sum_c W2[d, (h,c)] * att_src2[h,c]; same for dst
    W2r = W2.reshape(HID, H, OUT)
    w2s_t = np.einsum("dhc,hc->dh", W2r, a2s)
    w2d_t = np.einsum("dhc,hc->dh", W2r, a2d)
    # W2stack[(h,d), o] with 1/8 head-mean folded in; halves side by side
    w2stk = (W2r.transpose(1, 0, 2).reshape(H * HID, OUT) / 8.0)
    w2stk2 = np.concatenate([w2stk[0:128], w2stk[128:256]], axis=1)  # [128, 64]

    W1r = W1.reshape(D, H, HID)
    w1s_t = np.einsum("dhc,hc->dh", W1r, a1s.reshape(H, HID))
    w1d_t = np.einsum("dhc,hc->dh", W1r, a1d.reshape(H, HID))
    w1x = np.concatenate([W1, w1s_t, w1d_t], axis=1)  # [D, 272]
    common = {
        "embT": emb_pad.T.astype(bf).copy(),
        "w1x": w1x.astype(bf),
        "b1": np.tile(b1, (P, 1)),
        "b2": np.tile(b2, (P, 1)),
        "w2s": w2s_t.astype(bf),
        "w2d": w2d_t.astype(bf),
        "w2stk": w2stk2.astype(np.float32),
    }

    # per-node vocab ids by (slot, pos)
    xn_all = np.zeros((NC, RPC), np.int64)
    nodes = np.arange(N)
    xn_all[core_of[nodes // P], slot_of[nodes // P] * P + nodes % P] = x[nodes]

    in_maps = []
    out_perms = []
    e1i = np.arange(TOTE1)
    e2i = np.arange(TOTE2)
    for c in range(NC):
        idx1, dl1, idx2, dl2 = per_core[c]
        oh1 = np.zeros((P, TOTCH1, P), np.uint8)
        r1 = dl1 >= 0
        oh1[e1i[r1] % P, e1i[r1] // P, dl1[r1]] = 0x38
        oht1 = np.ascontiguousarray(oh1.transpose(2, 1, 0))
        oh2 = np.zeros((P, TOTCH2, P), np.uint8)
        r2 = dl2 >= 0
        oh2[e2i[r2] % P, e2i[r2] // P, dl2[r2]] = 0x38
        oht2 = np.ascontiguousarray(oh2.transpose(2, 1, 0))

        gi0, gi1, gch, gsz, perm = dec_core[c]
        di0 = np.concatenate([np.pad(gi0[g], (0, (dgc[g] - gch[g]) * P))
                              for g in range(4)])
        di1 = np.concatenate([np.pad(gi1[g], (0, (dgc[g] - gch[g]) * P))
                              for g in range(4)])
        pm = np.concatenate([np.pad(perm[g], (0, (dgc[g] - gch[g]) * P),
                                    constant_values=-1) for g in range(4)])
        out_perms.append(pm)

        # per-call exact counts (this core); guard all-pad calls with one
        # valid idx 0 so the ucode never sees num_idxs_reg == 0
        cnt1 = meta["cnt1"]
        cnt2 = meta["cnt2"]
        ch1_off = meta["ch1_off"]
        ch2_off = meta["ch2_off"]
        dgoff = np.concatenate([[0], np.cumsum(dgc)])[:-1]
        gcv = []
        for kind, b_, hh_, s_, cw_ in _call_plan(meta):
            if kind == "l1":
                real = int(min(max(cnt1[c, b_] - s_ * P, 0), cw_ * P))
                pos = (int(ch1_off[b_]) + s_) * P
                arr = idx1
            elif kind == "l2":
                real = int(min(max(cnt2[c, b_, hh_] - s_ * P, 0), cw_ * P))
                pos = (int(ch2_off[b_, hh_]) + s_) * P
                arr = idx2
            elif kind == "d0":
                real = int(min(max(gsz[b_] - s_ * P, 0), cw_ * P))
                pos = (int(dgoff[b_]) + s_) * P
                arr = di0
            else:
                real = int(min(max(gsz[b_] - s_ * P, 0), cw_ * P))
                pos = (int(dgoff[b_]) + s_) * P
                arr = di1
            if real == 0:
                real = 1
            gcv.append(real)
        gc = np.tile(np.asarray(gcv, np.int32), (P, 1))

        m = dict(common)
        m["gc"] = gc
        m["idx1"] = _wrap16(idx1.astype(np.int16))
        m["idx2"] = _wrap16(idx2.astype(np.int16))
        m["xn"] = _wrap16(xn_all[c].astype(np.int16))
        m["oh1"] = oh1.reshape(P, TOTE1).view(ml_dtypes.float8_e4m3)
        m["oht1"] = oht1.reshape(P, TOTE1).view(ml_dtypes.float8_e4m3)
        m["oh2"] = oh2.reshape(P, TOTE2).view(ml_dtypes.float8_e4m3)
        m["oht2"] = oht2.reshape(P, TOTE2).view(ml_dtypes.float8_e4m3)
        m["di0"] = _wrap16(di0.astype(np.int16))
        m["di1"] = _wrap16(di1.astype(np.int16))
        in_maps.append(m)

    return in_maps, out_perms, meta


def kernel(**inputs):
    in_maps, out_perms, meta = _make_inputs(inputs)
    nc = _build_nc(meta)
    import os
    trace = bool(int(os.environ.get("GAT_TRACE", "0")))
    if trace:
        try:
            import sys as _sys, types as _types
            import antenv as _antenv
            from trn_agent_boot.trn_boot import _ntff_profile_via_ctypes as _np_hook
            _hm = _types.ModuleType("antenv.axon_hooks")
            _hm.get_axon_ntff_profile_hook = (
                lambda: _np_hook('/opt/axon/libaxon_pjrt.so'))
            _hm.set_axon_ntff_profile_hook = lambda h: None
            _sys.modules["antenv.axon_hooks"] = _hm
            _antenv.axon_hooks = _hm
        except Exception:
            trace = False
    r = run_bass_kernel_spmd(nc, in_maps, core_ids=list(range(NC)), trace=trace)
    if trace and r.exec_time_ns:
        print("HW exec time: %d ns" % r.exec_time_ns)
        if r.per_core_scope_times:
            for s, mm in sorted(r.per_core_scope_times.items()):
                print("  scope %-8s %s" % (s, {k: "%dus" % (v // 1000) for k, v in mm.items()}))
        if r.instructions_and_trace:
            print("trace:", r.instructions_and_trace[1])

    out = np.zeros(EL, np.float32)
    for c in range(NC):
        resv = r.results[c]["out"]
        pm = out_perms[c]
        vals = resv.T.reshape(-1)
        valid = pm >= 0
        out[c * ELC + pm[valid]] = vals[valid]
    return out


if __name__ == "__main__":
    d = np.load("/root/problem/ref_data.npz")
    inputs = {k: d[k] for k in ("x", "edge_index", "edge_label_index", "emb",
                                "W1", "att_src1", "att_dst1", "b1",
                                "W2", "att_src2", "att_dst2", "b2")}
    got = kernel(**inputs)
    exp = d["expected"]
    denom = np.abs(exp).mean()
    rel = np.abs(got - exp) / denom
    print("Relative error: max %.3e mean %.3e" % (rel.max(), rel.mean()))
